# revision 25
# baseline (speedup 1.0000x reference)
"""GCN message-passing kernel for Trainium2, sharded over 8 NeuronCores.

Strategy (edge-cut partitioning per the sharding hint):
- Nodes renumbered so core c owns a contiguous block of 12544 (12500 real
  + 44 pad) destination nodes. Each edge is assigned to the core owning
  its destination; per core, edges are sorted by (source window of 32768
  rows, destination tile of 128 nodes) and padded so every (window, tile)
  segment is a whole number of 128-edge chunks, with identical chunk
  counts across cores (SPMD: one program, per-core data).
- Per layer: a bf16 feature table (linear + symmetric-norm source factor
  folded in, padded to 128-wide rows = the 256B SWDGE gather granule) is
  available on every core; per-edge messages are bulk-gathered with the
  SWDGE dma_gather ucode round-robined over 4 SWDGE queues (descriptor
  generation is the bottleneck and parallelizes across queues); the
  scatter-add is a bf16 PE matmul with a per-chunk one-hot selection
  matrix (built on DVE from destination indices), accumulating in PSUM
  and an f32 SBUF accumulator. dis[col] is applied to the accumulated
  rows, so no per-edge norm data is ever needed.
- Layer-1's table is built replicated (x is replicated); layer-2's table
  is built locally from the layer-1 aggregate and exchanged with one
  bf16 AllGather (which also serves as the halo exchange); pooled sums
  use one tiny AllReduce; the final linear + softmax runs replicated.
- A zero-byte warmup AllGather issues first so the one-time collective
  rendezvous cost overlaps with layer-1 compute.
"""

import sys

for _p in ("/opt/trn_rl_repo",):
    if _p not in sys.path:
        sys.path.insert(0, _p)

import numpy as np

N = 100000
E0 = 3200000
D_IN = 128
H1 = 32
H2 = 64
NCLS = 10
NG = 64
NCORES = 8
NLOC_REAL = 12500
NLOC = 12544          # per-core padded node count (98 * 128)
NP = NLOC * NCORES    # 100352 padded total
TLOC = NLOC // 128    # 98 dest tiles per core
WIN = 32768
NWIN = (NP + WIN - 1) // WIN  # 4
NTILES_G = NP // 128  # 784 global node tiles
CALL_CHUNKS = 64      # max 128-edge chunks per dma_gather call


def _host_prep(x, edge_index, batch):
    import ml_dtypes

    x = np.asarray(x, np.float32)
    ei = np.asarray(edge_index)
    batch = np.asarray(batch)

    # self loops participate in deg but are computed locally, not gathered
    row = ei[0].astype(np.int64)
    col = ei[1].astype(np.int64)
    deg = (np.bincount(row, minlength=N) + 1).astype(np.float32)  # +1 self loop

    # renumber: old g -> core c = g // 12500, new = c*NLOC + g % 12500
    def newid(g):
        return (g // NLOC_REAL) * NLOC + (g % NLOC_REAL)

    nrow = newid(row)
    ncol = newid(col)

    deg_new = np.ones(NP, np.float32)
    deg_new[newid(np.arange(N))] = deg

    # x transposed, padded
    xp = np.zeros((NP, D_IN), np.float32)
    xp[newid(np.arange(N))] = x
    xT = np.ascontiguousarray(xp.T)  # [128, NP]

    degT = np.ascontiguousarray(deg_new.reshape(NTILES_G, 128).T)  # [128, 784]

    cnt = np.bincount(np.asarray(batch, np.int64), minlength=NG).astype(np.float32)
    cnt = np.maximum(cnt, 1.0).reshape(NG, 1)

    # per-core edge structures
    ecore = ncol // NLOC
    percore = []
    for c in range(NCORES):
        m = ecore == c
        cr = nrow[m]
        cc = ncol[m] - c * NLOC
        w = (cr // WIN).astype(np.int64)
        t = (cc >> 7).astype(np.int64)
        dr = (cc & 127).astype(np.int64)
        order = np.lexsort((t, w))
        percore.append((cr[order], w[order], t[order], dr[order]))

    # segment chunk counts K[w][t], unified across cores
    K = np.zeros((NWIN, TLOC), np.int64)
    seg_counts = []
    for c in range(NCORES):
        cr, w, t, dr = percore[c]
        key = w * TLOC + t
        cnts = np.bincount(key, minlength=NWIN * TLOC).reshape(NWIN, TLOC)
        seg_counts.append(cnts)
        K = np.maximum(K, (cnts + 127) // 128)

    NCH = int(K.sum())
    EPAD = NCH * 128

    # per-core padded idx (window-relative) and dest_rel arrays
    idxw_list, drp_list = [], []
    for c in range(NCORES):
        cr, w, t, dr = percore[c]
        cnts = seg_counts[c]
        idx_flat = np.zeros(EPAD, np.int16)
        dr_flat = np.full(EPAD, -1.0, np.float32)
        pos_out = 0
        pos_in = 0
        for wi in range(NWIN):
            for ti in range(TLOC):
                n = int(cnts[wi, ti])
                kk = int(K[wi, ti]) * 128
                if kk == 0:
                    assert n == 0
                    continue
                idx_flat[pos_out:pos_out + n] = (cr[pos_in:pos_in + n] - wi * WIN).astype(np.int16)
                dr_flat[pos_out:pos_out + n] = dr[pos_in:pos_in + n].astype(np.float32)
                pos_in += n
                pos_out += kk
        assert pos_in == len(cr) and pos_out == EPAD

        wrapped = idx_flat.reshape(EPAD // 16, 16).T.copy()  # [16, EPAD//16]
        idxw_list.append(np.tile(wrapped, (8, 1)))            # [128, EPAD//16]
        drp_list.append(np.ascontiguousarray(
            dr_flat.reshape(NCH, 128).T).astype(ml_dtypes.bfloat16))  # [128, NCH]

    # segments (w, t, q0, nk) and gather calls (w, q_start, nq, col0)
    segments = []
    q = 0
    for wi in range(NWIN):
        for ti in range(TLOC):
            nk = int(K[wi, ti])
            while nk > 0:  # split oversized segments to fit one gather call
                piece = min(nk, CALL_CHUNKS)
                segments.append((wi, ti, q, piece))
                q += piece
                nk -= piece
    assert q == NCH

    calls = []
    cur = None
    for (wi, ti, q0, nk) in segments:
        if cur is not None and cur[0] == wi and cur[2] + nk <= CALL_CHUNKS:
            cur[2] += nk
            cur[3].append((ti, q0, nk))
        else:
            if cur is not None:
                calls.append(cur)
            cur = [wi, q0, nk, [(ti, q0, nk)]]
    if cur is not None:
        calls.append(cur)

    # per-core local metadata
    degL_list, bo_list = [], []
    batch64 = np.asarray(batch, np.int64)
    for c in range(NCORES):
        dl = deg_new[c * NLOC:(c + 1) * NLOC]
        degL_list.append(np.ascontiguousarray(dl.reshape(TLOC, 128).T))
        bo = np.full(NLOC, -1.0, np.float32)
        g0 = c * NLOC_REAL
        bo[:NLOC_REAL] = batch64[g0:g0 + NLOC_REAL].astype(np.float32)
        bo_list.append(np.ascontiguousarray(bo.reshape(TLOC, 128).T))

    xTloc_list = [np.ascontiguousarray(xT[:, c * NLOC:(c + 1) * NLOC])
                  for c in range(NCORES)]

    return dict(xT=xT, degT=degT, cnt=cnt, idxw=idxw_list, drp=drp_list,
                degL=degL_list, bo=bo_list, xTloc=xTloc_list, NCH=NCH,
                segments=segments, calls=calls)


def build_in_maps(prep, W1, b1, W2, b2, Wf, bf):
    import ml_dtypes

    ramp = np.tile(np.arange(128, dtype=np.float32), (128, 1))
    ident = np.eye(128, dtype=np.float32)
    common = dict(
        xT=prep["xT"], degT=prep["degT"], cnt=prep["cnt"],
        w1t=np.ascontiguousarray(
            np.asarray(W1, np.float32).T).astype(ml_dtypes.bfloat16),
        b1c=np.asarray(b1, np.float32).reshape(H1, 1),
        w2t=np.ascontiguousarray(np.asarray(W2, np.float32).T),
        b2c=np.asarray(b2, np.float32).reshape(H2, 1),
        wft=np.ascontiguousarray(np.asarray(Wf, np.float32).T),
        bfc=np.asarray(bf, np.float32).reshape(NCLS, 1),
        ramp=ramp, ident=ident,
    )
    in_maps = []
    for c in range(NCORES):
        m = dict(common)
        m["idxw"] = prep["idxw"][c]
        m["drp"] = prep["drp"][c]
        m["degL"] = prep["degL"][c]
        m["bo"] = prep["bo"][c]
        m["xTloc"] = prep["xTloc"][c]
        in_maps.append(m)
    return in_maps


def _win_len(w):
    return min(WIN, NP - w * WIN)


def _build_program(NCH, segments, calls):
    import concourse.bacc as bacc
    import concourse.mybir as mybir
    import concourse.tile as tile
    from concourse import library_config

    f32 = mybir.dt.float32
    bf16 = mybir.dt.bfloat16
    i16 = mybir.dt.int16
    AF = mybir.ActivationFunctionType
    OP = mybir.AluOpType

    nc = bacc.Bacc("TRN2", target_bir_lowering=False, debug=False,
                   num_devices=NCORES, num_swdge_queues=4)

    # I/O
    xT = nc.dram_tensor("xT", [128, NP], f32, kind="ExternalInput")
    degT = nc.dram_tensor("degT", [128, NTILES_G], f32, kind="ExternalInput")
    degL = nc.dram_tensor("degL", [128, TLOC], f32, kind="ExternalInput")
    idxw = nc.dram_tensor("idxw", [128, NCH * 8], i16, kind="ExternalInput")
    drp = nc.dram_tensor("drp", [128, NCH], bf16, kind="ExternalInput")
    bo = nc.dram_tensor("bo", [128, TLOC], f32, kind="ExternalInput")
    xTloc = nc.dram_tensor("xTloc", [128, NLOC], f32, kind="ExternalInput")
    cntd = nc.dram_tensor("cnt", [NG, 1], f32, kind="ExternalInput")
    w1t = nc.dram_tensor("w1t", [D_IN, H1], bf16, kind="ExternalInput")
    b1c = nc.dram_tensor("b1c", [H1, 1], f32, kind="ExternalInput")
    w2t = nc.dram_tensor("w2t", [H1, H2], f32, kind="ExternalInput")
    b2c = nc.dram_tensor("b2c", [H2, 1], f32, kind="ExternalInput")
    wft = nc.dram_tensor("wft", [H2, NCLS], f32, kind="ExternalInput")
    bfc = nc.dram_tensor("bfc", [NCLS, 1], f32, kind="ExternalInput")
    rampd = nc.dram_tensor("ramp", [128, 128], f32, kind="ExternalInput")
    identd = nc.dram_tensor("ident", [128, 128], f32, kind="ExternalInput")
    y = nc.dram_tensor("y", [NG, NCLS], f32, kind="ExternalOutput")

    # internal DRAM: bf16 tables with 256B (=128 elem) rows for the gather.
    # tab1 is split per source window so window-w gathers only depend on
    # the table writes of that window (starts layer-1 gathers ~4x earlier).
    tab1w = [nc.dram_tensor(f"tab1w{w}", [_win_len(w), 128], bf16)
             for w in range(NWIN)]
    tab2in = nc.dram_tensor("tab2in", [NLOC, 128], bf16)
    tab2 = nc.dram_tensor("tab2", [NP, 128], bf16, addr_space="Shared")
    cc2_in = nc.dram_tensor("cc2_in", [NG, H2], f32)
    cc2_out = nc.dram_tensor("cc2_out", [NG, H2], f32, addr_space="Shared")
    wa_in = nc.dram_tensor("wa_in", [16, 16], f32)
    wa_out = nc.dram_tensor("wa_out", [128, 16], f32, addr_space="Shared")

    tab1wv = [t.ap().rearrange("(a p) f -> p a f", p=128) for t in tab1w]
    tab2v = tab2.ap().rearrange("(a p) f -> p a f", p=128)    # [128, 784, 128]
    tab2inv = tab2in.ap().rearrange("(a p) f -> p a f", p=128)  # [128, 98, 128]
    # per-window gather sources
    src1 = [t.ap() for t in tab1w]
    src2 = [tab2.ap()[w * WIN:w * WIN + _win_len(w), :] for w in range(NWIN)]

    rg = [list(range(NCORES))]

    import os
    stages = os.environ.get("GNN_STAGES", "BCDFG")
    aggmode = os.environ.get("GNN_AGGMODE", "full")
    warmcc = os.environ.get("GNN_WARMCC", "1") == "1"

    with tile.TileContext(nc) as tc:
        nc.gpsimd.load_library(library_config.mlp)

        with tc.tile_pool(name="const", bufs=1) as cpool:
            if warmcc:
                nc.gpsimd.collective_compute(
                    "AllGather", mybir.AluOpType.bypass, replica_groups=rg,
                    ins=[wa_in.ap().opt()], outs=[wa_out.ap().opt()])
            ramp = cpool.tile([128, 128], f32)
            nc.sync.dma_start(out=ramp[:], in_=rampd[:])
            rampb = cpool.tile([128, 128], bf16)
            nc.vector.tensor_copy(rampb[:], ramp[:])
            ident = cpool.tile([128, 128], f32)
            nc.sync.dma_start(out=ident[:], in_=identd[:])
            identb = cpool.tile([128, 128], bf16)
            nc.vector.tensor_copy(identb[:], ident[:])
            drt = cpool.tile([128, NCH], bf16)
            nc.sync.dma_start(out=drt[:], in_=drp[:])
            w1s = cpool.tile([D_IN, H1], bf16)
            nc.sync.dma_start(out=w1s[:], in_=w1t[:])
            b1s = cpool.tile([H1, 1], f32)
            nc.sync.dma_start(out=b1s[:], in_=b1c[:])
            w2s = cpool.tile([H1, H2], f32)
            nc.sync.dma_start(out=w2s[:], in_=w2t[:])
            b2s = cpool.tile([H2, 1], f32)
            nc.sync.dma_start(out=b2s[:], in_=b2c[:])
            wfs = cpool.tile([H2, NCLS], f32)
            nc.sync.dma_start(out=wfs[:], in_=wft[:])
            bfs = cpool.tile([NCLS, 1], f32)
            nc.sync.dma_start(out=bfs[:], in_=bfc[:])
            cnts = cpool.tile([NG, 1], f32)
            nc.sync.dma_start(out=cnts[:], in_=cntd[:])
            bos = cpool.tile([128, TLOC], f32)
            nc.sync.dma_start(out=bos[:], in_=bo[:])

            # dis = deg ** -0.5 (global, tiled) and local
            dgt = cpool.tile([128, NTILES_G], f32)
            nc.sync.dma_start(out=dgt[:], in_=degT[:])
            dis = cpool.tile([128, NTILES_G], f32)
            nc.vector.reciprocal(dis[:], dgt[:])
            nc.scalar.activation(dis[:], dis[:], AF.Sqrt)
            dglt = cpool.tile([128, TLOC], f32)
            nc.sync.dma_start(out=dglt[:], in_=degL[:])
            disl = cpool.tile([128, TLOC], f32)
            nc.vector.reciprocal(disl[:], dglt[:])
            nc.scalar.activation(disl[:], disl[:], AF.Sqrt)

            acc1 = cpool.tile([128, TLOC * H1], f32)
            acc2 = cpool.tile([128, TLOC * H2], f32)

            def build_table1():
                with tc.tile_pool(name="tb1", bufs=3) as pool, \
                     tc.tile_pool(name="tb1p", bufs=2, space="PSUM") as pp, \
                     tc.tile_pool(name="tb1q", bufs=4, space="PSUM") as pq:
                    for i4 in range(NTILES_G // 4):
                        xt = pool.tile([128, 512], f32, tag="xt")
                        nc.sync.dma_start(out=xt[:], in_=xT[:, i4 * 512:(i4 + 1) * 512])
                        xtb = pool.tile([128, 512], bf16, tag="xtb")
                        nc.vector.tensor_copy(xtb[:], xt[:])
                        pt = pp.tile([H1, 512], f32, tag="pt")
                        nc.tensor.matmul(pt[:], w1s[:], xtb[:], start=True, stop=True)
                        hb = pool.tile([H1, 512], f32, tag="hb")
                        nc.vector.tensor_scalar(hb[:], pt[:], b1s[:], None, OP.add)
                        tt = pool.tile([128, 4, H1], bf16, tag="tt")
                        for j in range(4):
                            g = i4 * 4 + j
                            pj = pq.tile([128, H1], f32, tag="pj")
                            nc.tensor.transpose(pj[:], hb[:, j * 128:(j + 1) * 128],
                                                ident[:H1, :H1])
                            nc.vector.tensor_scalar(tt[:, j, :], pj[:],
                                                    dis[:, g:g + 1], None, OP.mult)
                        wv = i4 // 64  # 64 i4-groups (=256 tiles) per window
                        a0 = (i4 * 4) % 256
                        nc.sync.dma_start(out=tab1wv[wv][:, a0:a0 + 4, 0:H1],
                                          in_=tt[:])

            def init_acc1():
                # self-loop contribution: acc1[d] = dis_d * (x_d @ W1.T + b1)
                with tc.tile_pool(name="ia", bufs=3) as pool, \
                     tc.tile_pool(name="iap", bufs=2, space="PSUM") as pp, \
                     tc.tile_pool(name="iaq", bufs=2, space="PSUM") as pq:
                    for t in range(TLOC):
                        xt = pool.tile([128, 128], f32, tag="xt")
                        nc.sync.dma_start(out=xt[:],
                                          in_=xTloc[:, t * 128:(t + 1) * 128])
                        xtb = pool.tile([128, 128], bf16, tag="xtb")
                        nc.vector.tensor_copy(xtb[:], xt[:])
                        pt = pp.tile([H1, 128], f32, tag="pt")
                        nc.tensor.matmul(pt[:], w1s[:], xtb[:], start=True,
                                         stop=True)
                        hb = pool.tile([H1, 128], f32, tag="hb")
                        nc.vector.tensor_scalar(hb[:], pt[:], b1s[:], None, OP.add)
                        pj = pq.tile([128, H1], f32, tag="pj")
                        nc.tensor.transpose(pj[:], hb[:], ident[:H1, :H1])
                        nc.vector.tensor_scalar(acc1[:, t * H1:(t + 1) * H1],
                                                pj[:], disl[:, t:t + 1], None,
                                                OP.mult)

            # final segment-piece of each dest tile (for tile_cb interleave)
            _lastp = {}
            for (wi, ti, q0, nk) in segments:
                _lastp[ti] = (wi, q0)
            final_piece = {(v[0], k, v[1]) for k, v in _lastp.items()}

            def aggregate(wsrcs, Fl, acc, tile_cb=None, cb_pools=()):
                with tc.tile_pool(name="agg", bufs=4) as pool, \
                     tc.tile_pool(name="aggi", bufs=4) as ipool, \
                     tc.tile_pool(name="aggp", bufs=4, space="PSUM") as pp:
                    for ci, (wi, qs, nq, segs) in enumerate(calls):
                        it = ipool.tile([128, nq * 8], i16, tag="it")
                        nc.sync.dma_start(out=it[:],
                                          in_=idxw[:, qs * 8:(qs + nq) * 8])
                        gb = pool.tile([128, CALL_CHUNKS, 128], bf16, tag="gb")
                        nc.gpsimd.dma_gather(
                            gb[:, 0:nq, :], wsrcs[wi],
                            it[:], nq * 128, nq * 128, 128, single_packet=False,
                            queue_num=ci % 4)
                        if aggmode == "gather":
                            continue
                        for (ti, q0, nk) in segs:
                            ps = pp.tile([128, Fl], f32, tag="ps")
                            if aggmode == "full":
                                # batched one-hot build: S_all[p, k, d] =
                                # (ramp[p, d] == dest_rel[p, q0+k])
                                S = pool.tile([128, nk, 128], bf16, tag="S")
                                ramp_b = rampb[:].unsqueeze(1).broadcast_to(
                                    [128, nk, 128])
                                dr_b = drt[:, q0:q0 + nk].unsqueeze(2).broadcast_to(
                                    [128, nk, 128])
                                nc.vector.tensor_tensor(S[:], ramp_b, dr_b,
                                                        OP.is_equal)
                            for k in range(nk):
                                slot = q0 + k - qs
                                lhs = rampb[:] if aggmode == "noS" else S[:, k, :]
                                nc.tensor.matmul(ps[:], lhs, gb[:, slot, 0:Fl],
                                                 start=(k == 0), stop=(k == nk - 1),
                                                 skip_group_check=True)
                            nc.vector.tensor_tensor(
                                acc[:, ti * Fl:(ti + 1) * Fl],
                                acc[:, ti * Fl:(ti + 1) * Fl], ps[:], OP.add)
                            if tile_cb is not None and \
                                    (wi, ti, q0) in final_piece:
                                tile_cb(ti, *cb_pools)

            def aggregate_sb(wviews, Fl, acc):
                # SBUF-source gather: stage each 32768-token window of the
                # bf16 table into SBUF (bulk DMA), then transpose-mode
                # dma_gather (SBUF source avoids the HBM random-read
                # latency; descriptor gen round-robins 4 SWDGE queues).
                # Output is feature-major [token_elem, edge]; a per-chunk
                # PE transpose restores edge-major for the scatter matmul.
                RKS = WIN // 128  # 256 ranks per full window
                with tc.tile_pool(name="agw", bufs=1) as wpool, \
                     tc.tile_pool(name="agg", bufs=3) as pool, \
                     tc.tile_pool(name="aggi", bufs=4) as ipool, \
                     tc.tile_pool(name="aggs", bufs=2) as spool, \
                     tc.tile_pool(name="aggt", bufs=3) as tpool, \
                     tc.tile_pool(name="aggp", bufs=4, space="PSUM") as pp, \
                     tc.tile_pool(name="aggq", bufs=4, space="PSUM") as pq:
                    cur_w = -1
                    win = None
                    for ci, (wi, qs, nq, segs) in enumerate(calls):
                        if wi != cur_w:
                            cur_w = wi
                            nrk = _win_len(wi) // 128
                            win = wpool.tile([128, RKS, 128], bf16, tag="win")
                            nc.sync.dma_start(
                                out=win[:, 0:nrk, :],
                                in_=wviews[wi][:, 0:nrk, :])
                        it = ipool.tile([128, nq * 8], i16, tag="it")
                        nc.sync.dma_start(out=it[:],
                                          in_=idxw[:, qs * 8:(qs + nq) * 8])
                        gb = pool.tile([128, 1, CALL_CHUNKS * 128], bf16,
                                       tag="gb")
                        nc.gpsimd.dma_gather(
                            gb[:, :, 0:nq * 128], win[:],
                            it[:], nq * 128, nq * 128, 128,
                            transpose=True, single_packet=False,
                            queue_num=ci % 4,
                            sbuf_tokens_per_rank=128,
                            sbuf_free_dim_per_rank=256,
                        )
                        if aggmode == "gather":
                            continue
                        for (ti, q0, nk) in segs:
                            ps = pp.tile([128, Fl], f32, tag="ps")
                            if aggmode == "full":
                                S = spool.tile([128, nk, 128], bf16, tag="S")
                                ramp_b = rampb[:].unsqueeze(1).broadcast_to(
                                    [128, nk, 128])
                                dr_b = drt[:, q0:q0 + nk].unsqueeze(2).broadcast_to(
                                    [128, nk, 128])
                                nc.vector.tensor_tensor(S[:], ramp_b, dr_b,
                                                        OP.is_equal)
                            for k in range(nk):
                                slot = q0 + k - qs
                                tp = pq.tile([128, 128], bf16, tag="tp")
                                nc.tensor.transpose(
                                    tp[:], gb[:, 0, slot * 128:(slot + 1) * 128],
                                    identb[:])
                                ts = tpool.tile([128, Fl], bf16, tag="ts")
                                nc.scalar.activation(ts[:], tp[:, 0:Fl], AF.Copy)
                                lhs = rampb[:] if aggmode == "noS" else S[:, k, :]
                                nc.tensor.matmul(ps[:], lhs, ts[:],
                                                 start=(k == 0),
                                                 stop=(k == nk - 1),
                                                 skip_group_check=True)
                            nc.vector.tensor_tensor(
                                acc[:, ti * Fl:(ti + 1) * Fl],
                                acc[:, ti * Fl:(ti + 1) * Fl], ps[:], OP.add)

            def d_tile(t, pool, pp, pq):
                # h1 = relu(dis*acc1); tab2_local[d] = dis_d*(h1_d @ W2.T + b2)
                # (also the layer-2 self-loop init for acc2)
                rl = pool.tile([128, H1], f32, tag="rl")
                nc.scalar.activation(rl[:], acc1[:, t * H1:(t + 1) * H1],
                                     AF.Relu, scale=disl[:, t:t + 1])
                pT = pp.tile([H1, 128], f32, tag="pT")
                nc.tensor.transpose(pT[:], rl[:], ident[:, :])
                rT = pool.tile([H1, 128], f32, tag="rT")
                nc.scalar.activation(rT[:], pT[:], AF.Copy)
                p2 = pp.tile([H2, 128], f32, tag="p2")
                nc.tensor.matmul(p2[:], w2s[:], rT[:], start=True, stop=True)
                hb2 = pool.tile([H2, 128], f32, tag="hb2")
                nc.vector.tensor_scalar(hb2[:], p2[:], b2s[:], None, OP.add)
                pj2 = pq.tile([128, H2], f32, tag="pj2")
                nc.tensor.transpose(pj2[:], hb2[:], ident[:H2, :H2])
                nc.vector.tensor_scalar(acc2[:, t * H2:(t + 1) * H2],
                                        pj2[:], disl[:, t:t + 1], None, OP.mult)
                tb2 = pool.tile([128, H2], bf16, tag="tb2")
                nc.vector.tensor_copy(tb2[:], acc2[:, t * H2:(t + 1) * H2])
                nc.sync.dma_start(out=tab2inv[:, t, 0:H2], in_=tb2[:])

            def tab2_allgather():
                nc.gpsimd.collective_compute(
                    "AllGather", mybir.AluOpType.bypass, replica_groups=rg,
                    ins=[tab2in.ap().opt()], outs=[tab2.ap().opt()])

            def build_tab2_local_allgather():
                with tc.tile_pool(name="rt", bufs=3) as pool, \
                     tc.tile_pool(name="rtp", bufs=2, space="PSUM") as pp, \
                     tc.tile_pool(name="rtq", bufs=2, space="PSUM") as pq:
                    for t in range(TLOC):
                        d_tile(t, pool, pp, pq)
                tab2_allgather()

            def g_tile(t, pool, pps, first, last):
                # pooled-sum contribution of dest tile t (per-graph one-hot)
                r2 = pool.tile([128, H2], f32, tag="r2")
                nc.scalar.activation(r2[:], acc2[:, t * H2:(t + 1) * H2],
                                     AF.Relu, scale=disl[:, t:t + 1])
                Sb = pool.tile([128, NG], f32, tag="Sb")
                nc.vector.tensor_scalar(Sb[:], ramp[:, 0:NG],
                                        bos[:, t:t + 1], None, OP.is_equal)
                nc.tensor.matmul(pps[:], Sb[:], r2[:],
                                 start=first, stop=last,
                                 skip_group_check=True)

            # callback emission order (order in which tiles finalize)
            cb_order = [ti for (wi, ti, q0, nk) in segments
                        if (wi, ti, q0) in final_piece]
            assert len(cb_order) == TLOC

            def pool_and_head(pps=None):
                with tc.tile_pool(name="hd", bufs=3) as pool, \
                     tc.tile_pool(name="hdp", bufs=1, space="PSUM") as pp:
                    if pps is None:
                        pps = pp.tile([NG, H2], f32, tag="pool")
                        for t in range(TLOC):
                            g_tile(t, pool, pps, t == 0, t == TLOC - 1)
                    pls = pool.tile([NG, H2], f32, tag="pls")
                    nc.scalar.activation(pls[:], pps[:], AF.Copy)
                    nc.sync.dma_start(out=cc2_in[:, :], in_=pls[:])
                    nc.gpsimd.collective_compute(
                        "AllReduce", OP.add, replica_groups=rg,
                        ins=[cc2_in.ap().opt()], outs=[cc2_out.ap().opt()])
                    psb = pool.tile([NG, H2], f32, tag="psb")
                    nc.sync.dma_start(out=psb[:], in_=cc2_out[:, :])
                    rc = pool.tile([NG, 1], f32, tag="rc")
                    nc.vector.reciprocal(rc[:], cnts[:])
                    mean = pool.tile([NG, H2], f32, tag="mean")
                    nc.vector.tensor_scalar(mean[:], psb[:], rc[:], None, OP.mult)
                    # transpose mean -> [H2, NG]
                    pmT = pp.tile([H2, NG], f32, tag="pmT")
                    nc.tensor.transpose(pmT[:], mean[:], ident[:NG, :NG])
                    meanT = pool.tile([H2, NG], f32, tag="meanT")
                    nc.scalar.activation(meanT[:], pmT[:], AF.Copy)
                    # logitsT [NCLS, NG]
                    plt = pp.tile([NCLS, NG], f32, tag="plt")
                    nc.tensor.matmul(plt[:], wfs[:], meanT[:], start=True, stop=True)
                    lts = pool.tile([NCLS, NG], f32, tag="lts")
                    nc.vector.tensor_scalar(lts[:], plt[:], bfs[:], None, OP.add)
                    # transpose -> [NG, NCLS]
                    plg = pp.tile([NG, NCLS], f32, tag="plg")
                    nc.tensor.transpose(plg[:], lts[:], ident[:NCLS, :NCLS])
                    lg = pool.tile([NG, NCLS], f32, tag="lg")
                    nc.scalar.activation(lg[:], plg[:], AF.Copy)
                    # softmax over free dim
                    mx = pool.tile([NG, 1], f32, tag="mx")
                    nc.vector.tensor_reduce(mx[:], lg[:], mybir.AxisListType.X,
                                            OP.max, negate=True)
                    ex = pool.tile([NG, NCLS], f32, tag="ex")
                    nc.scalar.activation(ex[:], lg[:], AF.Exp, bias=mx[:])
                    sm = pool.tile([NG, 1], f32, tag="sm")
                    nc.vector.tensor_reduce(sm[:], ex[:], mybir.AxisListType.X,
                                            OP.add)
                    rs = pool.tile([NG, 1], f32, tag="rs")
                    nc.vector.reciprocal(rs[:], sm[:])
                    yt = pool.tile([NG, NCLS], f32, tag="yt")
                    nc.vector.tensor_scalar(yt[:], ex[:], rs[:], None, OP.mult)
                    nc.sync.dma_start(out=y[:, :], in_=yt[:])

            sbg = os.environ.get("GNN_SBGATHER", "0") == "1"
            di = os.environ.get("GNN_DI", "1") == "1"
            sv2 = [tab2v[:, w * 256:w * 256 + _win_len(w) // 128, :]
                   for w in range(NWIN)]
            if "B" in stages:
                build_table1()
            interleaved_d = di and not sbg and "C" in stages and "D" in stages
            if "C" in stages:
                init_acc1()
                if sbg:
                    aggregate_sb(tab1wv, H1, acc1)
                elif interleaved_d:
                    # fold the layer-2 table build into the aggregation: each
                    # dest tile's tab2 row block is computed as soon as its
                    # accumulator finalizes, hiding D behind the gathers.
                    with tc.tile_pool(name="rt", bufs=3) as dpool, \
                         tc.tile_pool(name="rtp", bufs=1, space="PSUM") as dpp, \
                         tc.tile_pool(name="rtq", bufs=1, space="PSUM") as dpq:
                        aggregate(src1, H1, acc1, tile_cb=d_tile,
                                  cb_pools=(dpool, dpp, dpq))
                    tab2_allgather()
                else:
                    aggregate(src1, H1, acc1)
            if "D" in stages and not interleaved_d:
                build_tab2_local_allgather()
            gi = os.environ.get("GNN_GI", "1") == "1"
            interleaved_g = gi and not sbg and "F" in stages and "G" in stages \
                and aggmode == "full"
            gpps = None
            if "F" in stages:
                if sbg:
                    aggregate_sb(sv2, H2, acc2)
                elif interleaved_g:
                    # fold the graph-pooling matmul into the aggregation:
                    # each dest tile is pooled as soon as it finalizes, so
                    # only the AllReduce + head remain after the gathers.
                    with tc.tile_pool(name="gp", bufs=1, space="PSUM") as gpp, \
                         tc.tile_pool(name="gs", bufs=2) as gspool:
                        gpps = gpp.tile([NG, H2], f32, tag="pool")
                        cb_first, cb_last = cb_order[0], cb_order[-1]

                        def _gcb(ti, pool, pps):
                            g_tile(ti, pool, pps, ti == cb_first, ti == cb_last)

                        aggregate(src2, H2, acc2, tile_cb=_gcb,
                                  cb_pools=(gspool, gpps))
                        pool_and_head(gpps)
                else:
                    aggregate(src2, H2, acc2)
            if "G" in stages:
                if not interleaved_g:
                    pool_and_head()
            else:
                with tc.tile_pool(name="dbg", bufs=1) as dpool:
                    dt = dpool.tile([NG, NCLS], f32)
                    nc.vector.memset(dt[:], 0.0)
                    nc.sync.dma_start(out=y[:, :], in_=dt[:])

    nc.compile()
    return nc


def kernel(x, edge_index, batch, W1, b1, W2, b2, Wf, bf):
    from concourse.bass_utils import run_bass_kernel_spmd

    prep = _host_prep(x, edge_index, batch)
    nc = _build_program(prep["NCH"], prep["segments"], prep["calls"])
    in_maps = build_in_maps(prep, W1, b1, W2, b2, Wf, bf)
    res = run_bass_kernel_spmd(nc, in_maps, core_ids=list(range(NCORES)))
    return np.asarray(res.results[0]["y"], np.float32)


# revision 26
# speedup vs baseline: 1.0168x; 1.0168x over previous
"""GCN message-passing kernel for Trainium2, sharded over 8 NeuronCores.

Strategy (edge-cut partitioning per the sharding hint):
- Nodes renumbered so core c owns a contiguous block of 12544 (12500 real
  + 44 pad) destination nodes. Each edge is assigned to the core owning
  its destination; per core, edges are sorted by (source window of 32768
  rows, destination tile of 128 nodes) and padded so every (window, tile)
  segment is a whole number of 128-edge chunks, with identical chunk
  counts across cores (SPMD: one program, per-core data).
- Per layer: a bf16 feature table (linear + symmetric-norm source factor
  folded in, padded to 128-wide rows = the 256B SWDGE gather granule) is
  available on every core; per-edge messages are bulk-gathered with the
  SWDGE dma_gather ucode round-robined over 4 SWDGE queues (descriptor
  generation is the bottleneck and parallelizes across queues); the
  scatter-add is a bf16 PE matmul with a per-chunk one-hot selection
  matrix (built on DVE from destination indices), accumulating in PSUM
  and an f32 SBUF accumulator. dis[col] is applied to the accumulated
  rows, so no per-edge norm data is ever needed.
- Layer-1's table is built replicated (x is replicated); layer-2's table
  is built locally from the layer-1 aggregate and exchanged with one
  bf16 AllGather (which also serves as the halo exchange); pooled sums
  use one tiny AllReduce; the final linear + softmax runs replicated.
- A zero-byte warmup AllGather issues first so the one-time collective
  rendezvous cost overlaps with layer-1 compute.
"""

import sys

for _p in ("/opt/trn_rl_repo",):
    if _p not in sys.path:
        sys.path.insert(0, _p)

import numpy as np

N = 100000
E0 = 3200000
D_IN = 128
H1 = 32
H2 = 64
NCLS = 10
NG = 64
NCORES = 8
NLOC_REAL = 12500
NLOC = 12544          # per-core padded node count (98 * 128)
NP = NLOC * NCORES    # 100352 padded total
TLOC = NLOC // 128    # 98 dest tiles per core
WIN = 32768
NWIN = (NP + WIN - 1) // WIN  # 4
NTILES_G = NP // 128  # 784 global node tiles
CALL_CHUNKS = 64      # max 128-edge chunks per dma_gather call


def _host_prep(x, edge_index, batch):
    import ml_dtypes

    x = np.asarray(x, np.float32)
    ei = np.asarray(edge_index)
    batch = np.asarray(batch)

    # self loops participate in deg but are computed locally, not gathered
    row = ei[0].astype(np.int64)
    col = ei[1].astype(np.int64)
    deg = (np.bincount(row, minlength=N) + 1).astype(np.float32)  # +1 self loop

    # renumber: old g -> core c = g // 12500, new = c*NLOC + g % 12500
    def newid(g):
        return (g // NLOC_REAL) * NLOC + (g % NLOC_REAL)

    nrow = newid(row)
    ncol = newid(col)

    deg_new = np.ones(NP, np.float32)
    deg_new[newid(np.arange(N))] = deg

    # x transposed, padded
    xp = np.zeros((NP, D_IN), np.float32)
    xp[newid(np.arange(N))] = x
    xT = np.ascontiguousarray(xp.T)  # [128, NP]

    degT = np.ascontiguousarray(deg_new.reshape(NTILES_G, 128).T)  # [128, 784]

    cnt = np.bincount(np.asarray(batch, np.int64), minlength=NG).astype(np.float32)
    cnt = np.maximum(cnt, 1.0).reshape(NG, 1)

    # per-core edge structures
    ecore = ncol // NLOC
    percore = []
    for c in range(NCORES):
        m = ecore == c
        cr = nrow[m]
        cc = ncol[m] - c * NLOC
        w = (cr // WIN).astype(np.int64)
        t = (cc >> 7).astype(np.int64)
        dr = (cc & 127).astype(np.int64)
        order = np.lexsort((t, w))
        percore.append((cr[order], w[order], t[order], dr[order]))

    # segment chunk counts K[w][t], unified across cores
    K = np.zeros((NWIN, TLOC), np.int64)
    seg_counts = []
    for c in range(NCORES):
        cr, w, t, dr = percore[c]
        key = w * TLOC + t
        cnts = np.bincount(key, minlength=NWIN * TLOC).reshape(NWIN, TLOC)
        seg_counts.append(cnts)
        K = np.maximum(K, (cnts + 127) // 128)

    NCH = int(K.sum())
    EPAD = NCH * 128

    # per-core padded idx (window-relative) and dest_rel arrays
    idxw_list, drp_list = [], []
    for c in range(NCORES):
        cr, w, t, dr = percore[c]
        cnts = seg_counts[c]
        idx_flat = np.zeros(EPAD, np.int16)
        dr_flat = np.full(EPAD, -1.0, np.float32)
        pos_out = 0
        pos_in = 0
        for wi in range(NWIN):
            for ti in range(TLOC):
                n = int(cnts[wi, ti])
                kk = int(K[wi, ti]) * 128
                if kk == 0:
                    assert n == 0
                    continue
                idx_flat[pos_out:pos_out + n] = (cr[pos_in:pos_in + n] - wi * WIN).astype(np.int16)
                dr_flat[pos_out:pos_out + n] = dr[pos_in:pos_in + n].astype(np.float32)
                pos_in += n
                pos_out += kk
        assert pos_in == len(cr) and pos_out == EPAD

        wrapped = idx_flat.reshape(EPAD // 16, 16).T.copy()  # [16, EPAD//16]
        idxw_list.append(np.tile(wrapped, (8, 1)))            # [128, EPAD//16]
        drp_list.append(np.ascontiguousarray(
            dr_flat.reshape(NCH, 128).T).astype(ml_dtypes.bfloat16))  # [128, NCH]

    # segments (w, t, q0, nk) and gather calls (w, q_start, nq, col0)
    segments = []
    q = 0
    for wi in range(NWIN):
        for ti in range(TLOC):
            nk = int(K[wi, ti])
            while nk > 0:  # split oversized segments to fit one gather call
                piece = min(nk, CALL_CHUNKS)
                segments.append((wi, ti, q, piece))
                q += piece
                nk -= piece
    assert q == NCH

    calls = []
    cur = None
    for (wi, ti, q0, nk) in segments:
        if cur is not None and cur[0] == wi and cur[2] + nk <= CALL_CHUNKS:
            cur[2] += nk
            cur[3].append((ti, q0, nk))
        else:
            if cur is not None:
                calls.append(cur)
            cur = [wi, q0, nk, [(ti, q0, nk)]]
    if cur is not None:
        calls.append(cur)

    # per-core local metadata
    degL_list, bo_list = [], []
    batch64 = np.asarray(batch, np.int64)
    for c in range(NCORES):
        dl = deg_new[c * NLOC:(c + 1) * NLOC]
        degL_list.append(np.ascontiguousarray(dl.reshape(TLOC, 128).T))
        bo = np.full(NLOC, -1.0, np.float32)
        g0 = c * NLOC_REAL
        bo[:NLOC_REAL] = batch64[g0:g0 + NLOC_REAL].astype(np.float32)
        bo_list.append(np.ascontiguousarray(bo.reshape(TLOC, 128).T))

    xTloc_list = [np.ascontiguousarray(xT[:, c * NLOC:(c + 1) * NLOC])
                  for c in range(NCORES)]

    return dict(xT=xT, degT=degT, cnt=cnt, idxw=idxw_list, drp=drp_list,
                degL=degL_list, bo=bo_list, xTloc=xTloc_list, NCH=NCH,
                segments=segments, calls=calls)


def build_in_maps(prep, W1, b1, W2, b2, Wf, bf):
    import ml_dtypes

    ramp = np.tile(np.arange(128, dtype=np.float32), (128, 1))
    ident = np.eye(128, dtype=np.float32)
    common = dict(
        xT=prep["xT"], degT=prep["degT"], cnt=prep["cnt"],
        w1t=np.ascontiguousarray(
            np.asarray(W1, np.float32).T).astype(ml_dtypes.bfloat16),
        b1c=np.asarray(b1, np.float32).reshape(H1, 1),
        w2t=np.ascontiguousarray(np.asarray(W2, np.float32).T),
        b2c=np.asarray(b2, np.float32).reshape(H2, 1),
        wft=np.ascontiguousarray(np.asarray(Wf, np.float32).T),
        bfc=np.asarray(bf, np.float32).reshape(NCLS, 1),
        ramp=ramp, ident=ident,
    )
    in_maps = []
    for c in range(NCORES):
        m = dict(common)
        m["idxw"] = prep["idxw"][c]
        m["drp"] = prep["drp"][c]
        m["degL"] = prep["degL"][c]
        m["bo"] = prep["bo"][c]
        m["xTloc"] = prep["xTloc"][c]
        in_maps.append(m)
    return in_maps


def _win_len(w):
    return min(WIN, NP - w * WIN)


def _build_program(NCH, segments, calls):
    import concourse.bacc as bacc
    import concourse.mybir as mybir
    import concourse.tile as tile
    from concourse import library_config

    f32 = mybir.dt.float32
    bf16 = mybir.dt.bfloat16
    i16 = mybir.dt.int16
    AF = mybir.ActivationFunctionType
    OP = mybir.AluOpType

    nc = bacc.Bacc("TRN2", target_bir_lowering=False, debug=False,
                   num_devices=NCORES, num_swdge_queues=4)

    # I/O
    xT = nc.dram_tensor("xT", [128, NP], f32, kind="ExternalInput")
    degT = nc.dram_tensor("degT", [128, NTILES_G], f32, kind="ExternalInput")
    degL = nc.dram_tensor("degL", [128, TLOC], f32, kind="ExternalInput")
    idxw = nc.dram_tensor("idxw", [128, NCH * 8], i16, kind="ExternalInput")
    drp = nc.dram_tensor("drp", [128, NCH], bf16, kind="ExternalInput")
    bo = nc.dram_tensor("bo", [128, TLOC], f32, kind="ExternalInput")
    xTloc = nc.dram_tensor("xTloc", [128, NLOC], f32, kind="ExternalInput")
    cntd = nc.dram_tensor("cnt", [NG, 1], f32, kind="ExternalInput")
    w1t = nc.dram_tensor("w1t", [D_IN, H1], bf16, kind="ExternalInput")
    b1c = nc.dram_tensor("b1c", [H1, 1], f32, kind="ExternalInput")
    w2t = nc.dram_tensor("w2t", [H1, H2], f32, kind="ExternalInput")
    b2c = nc.dram_tensor("b2c", [H2, 1], f32, kind="ExternalInput")
    wft = nc.dram_tensor("wft", [H2, NCLS], f32, kind="ExternalInput")
    bfc = nc.dram_tensor("bfc", [NCLS, 1], f32, kind="ExternalInput")
    rampd = nc.dram_tensor("ramp", [128, 128], f32, kind="ExternalInput")
    identd = nc.dram_tensor("ident", [128, 128], f32, kind="ExternalInput")
    y = nc.dram_tensor("y", [NG, NCLS], f32, kind="ExternalOutput")

    # internal DRAM: bf16 tables with 256B (=128 elem) rows for the gather.
    # tab1 is split per source window so window-w gathers only depend on
    # the table writes of that window (starts layer-1 gathers ~4x earlier).
    tab1w = [nc.dram_tensor(f"tab1w{w}", [_win_len(w), 128], bf16)
             for w in range(NWIN)]
    tab2in = nc.dram_tensor("tab2in", [NLOC, 128], bf16)
    tab2 = nc.dram_tensor("tab2", [NP, 128], bf16, addr_space="Shared")
    cc2_in = nc.dram_tensor("cc2_in", [NG, H2], f32)
    cc2_out = nc.dram_tensor("cc2_out", [NG, H2], f32, addr_space="Shared")
    wa_in = nc.dram_tensor("wa_in", [16, 16], f32)
    wa_out = nc.dram_tensor("wa_out", [128, 16], f32, addr_space="Shared")

    tab1wv = [t.ap().rearrange("(a p) f -> p a f", p=128) for t in tab1w]
    tab2v = tab2.ap().rearrange("(a p) f -> p a f", p=128)    # [128, 784, 128]
    tab2inv = tab2in.ap().rearrange("(a p) f -> p a f", p=128)  # [128, 98, 128]
    # per-window gather sources
    src1 = [t.ap() for t in tab1w]
    src2 = [tab2.ap()[w * WIN:w * WIN + _win_len(w), :] for w in range(NWIN)]

    rg = [list(range(NCORES))]

    import os
    stages = os.environ.get("GNN_STAGES", "BCDFG")
    aggmode = os.environ.get("GNN_AGGMODE", "full")
    warmcc = os.environ.get("GNN_WARMCC", "1") == "1"

    with tile.TileContext(nc) as tc:
        nc.gpsimd.load_library(library_config.mlp)

        with tc.tile_pool(name="const", bufs=1) as cpool:
            if warmcc:
                nc.gpsimd.collective_compute(
                    "AllGather", mybir.AluOpType.bypass, replica_groups=rg,
                    ins=[wa_in.ap().opt()], outs=[wa_out.ap().opt()])
            ramp = cpool.tile([128, 128], f32)
            nc.sync.dma_start(out=ramp[:], in_=rampd[:])
            rampb = cpool.tile([128, 128], bf16)
            nc.vector.tensor_copy(rampb[:], ramp[:])
            ident = cpool.tile([128, 128], f32)
            nc.sync.dma_start(out=ident[:], in_=identd[:])
            identb = cpool.tile([128, 128], bf16)
            nc.vector.tensor_copy(identb[:], ident[:])
            drt = cpool.tile([128, NCH], bf16)
            nc.sync.dma_start(out=drt[:], in_=drp[:])
            w1s = cpool.tile([D_IN, H1], bf16)
            nc.sync.dma_start(out=w1s[:], in_=w1t[:])
            b1s = cpool.tile([H1, 1], f32)
            nc.sync.dma_start(out=b1s[:], in_=b1c[:])
            w2s = cpool.tile([H1, H2], f32)
            nc.sync.dma_start(out=w2s[:], in_=w2t[:])
            b2s = cpool.tile([H2, 1], f32)
            nc.sync.dma_start(out=b2s[:], in_=b2c[:])
            wfs = cpool.tile([H2, NCLS], f32)
            nc.sync.dma_start(out=wfs[:], in_=wft[:])
            bfs = cpool.tile([NCLS, 1], f32)
            nc.sync.dma_start(out=bfs[:], in_=bfc[:])
            cnts = cpool.tile([NG, 1], f32)
            nc.sync.dma_start(out=cnts[:], in_=cntd[:])
            bos = cpool.tile([128, TLOC], f32)
            nc.sync.dma_start(out=bos[:], in_=bo[:])

            # dis = deg ** -0.5 (global, tiled) and local
            dgt = cpool.tile([128, NTILES_G], f32)
            nc.sync.dma_start(out=dgt[:], in_=degT[:])
            dis = cpool.tile([128, NTILES_G], f32)
            nc.vector.reciprocal(dis[:], dgt[:])
            nc.scalar.activation(dis[:], dis[:], AF.Sqrt)
            dglt = cpool.tile([128, TLOC], f32)
            nc.sync.dma_start(out=dglt[:], in_=degL[:])
            disl = cpool.tile([128, TLOC], f32)
            nc.vector.reciprocal(disl[:], dglt[:])
            nc.scalar.activation(disl[:], disl[:], AF.Sqrt)

            acc1 = cpool.tile([128, TLOC * H1], f32)
            acc2 = cpool.tile([128, TLOC * H2], f32)

            def build_table1():
                with tc.tile_pool(name="tb1", bufs=3) as pool, \
                     tc.tile_pool(name="tb1p", bufs=2, space="PSUM") as pp, \
                     tc.tile_pool(name="tb1q", bufs=4, space="PSUM") as pq:
                    for i4 in range(NTILES_G // 4):
                        xt = pool.tile([128, 512], f32, tag="xt")
                        nc.sync.dma_start(out=xt[:], in_=xT[:, i4 * 512:(i4 + 1) * 512])
                        xtb = pool.tile([128, 512], bf16, tag="xtb")
                        nc.vector.tensor_copy(xtb[:], xt[:])
                        pt = pp.tile([H1, 512], f32, tag="pt")
                        nc.tensor.matmul(pt[:], w1s[:], xtb[:], start=True, stop=True)
                        hb = pool.tile([H1, 512], f32, tag="hb")
                        nc.vector.tensor_scalar(hb[:], pt[:], b1s[:], None, OP.add)
                        tt = pool.tile([128, 4, H1], bf16, tag="tt")
                        for j in range(4):
                            g = i4 * 4 + j
                            pj = pq.tile([128, H1], f32, tag="pj")
                            nc.tensor.transpose(pj[:], hb[:, j * 128:(j + 1) * 128],
                                                ident[:H1, :H1])
                            nc.vector.tensor_scalar(tt[:, j, :], pj[:],
                                                    dis[:, g:g + 1], None, OP.mult)
                        wv = i4 // 64  # 64 i4-groups (=256 tiles) per window
                        a0 = (i4 * 4) % 256
                        nc.sync.dma_start(out=tab1wv[wv][:, a0:a0 + 4, 0:H1],
                                          in_=tt[:])

            def init_acc1():
                # self-loop contribution: acc1[d] = dis_d * (x_d @ W1.T + b1)
                with tc.tile_pool(name="ia", bufs=3) as pool, \
                     tc.tile_pool(name="iap", bufs=2, space="PSUM") as pp, \
                     tc.tile_pool(name="iaq", bufs=2, space="PSUM") as pq:
                    for t in range(TLOC):
                        xt = pool.tile([128, 128], f32, tag="xt")
                        nc.sync.dma_start(out=xt[:],
                                          in_=xTloc[:, t * 128:(t + 1) * 128])
                        xtb = pool.tile([128, 128], bf16, tag="xtb")
                        nc.vector.tensor_copy(xtb[:], xt[:])
                        pt = pp.tile([H1, 128], f32, tag="pt")
                        nc.tensor.matmul(pt[:], w1s[:], xtb[:], start=True,
                                         stop=True)
                        hb = pool.tile([H1, 128], f32, tag="hb")
                        nc.vector.tensor_scalar(hb[:], pt[:], b1s[:], None, OP.add)
                        pj = pq.tile([128, H1], f32, tag="pj")
                        nc.tensor.transpose(pj[:], hb[:], ident[:H1, :H1])
                        nc.vector.tensor_scalar(acc1[:, t * H1:(t + 1) * H1],
                                                pj[:], disl[:, t:t + 1], None,
                                                OP.mult)

            # final segment-piece of each dest tile (for tile_cb interleave)
            _lastp = {}
            for (wi, ti, q0, nk) in segments:
                _lastp[ti] = (wi, q0)
            final_piece = {(v[0], k, v[1]) for k, v in _lastp.items()}

            def aggregate(wsrcs, Fl, acc, tile_cb=None, cb_pools=()):
                qload = [0, 0, 0, 0]  # least-loaded SWDGE queue assignment
                with tc.tile_pool(name="agg", bufs=4) as pool, \
                     tc.tile_pool(name="aggi", bufs=4) as ipool, \
                     tc.tile_pool(name="aggp", bufs=4, space="PSUM") as pp:
                    for ci, (wi, qs, nq, segs) in enumerate(calls):
                        it = ipool.tile([128, nq * 8], i16, tag="it")
                        nc.sync.dma_start(out=it[:],
                                          in_=idxw[:, qs * 8:(qs + nq) * 8])
                        gb = pool.tile([128, CALL_CHUNKS, 128], bf16, tag="gb")
                        qn = qload.index(min(qload))
                        qload[qn] += nq
                        nc.gpsimd.dma_gather(
                            gb[:, 0:nq, :], wsrcs[wi],
                            it[:], nq * 128, nq * 128, 128, single_packet=False,
                            queue_num=qn)
                        if aggmode == "gather":
                            continue
                        for (ti, q0, nk) in segs:
                            ps = pp.tile([128, Fl], f32, tag="ps")
                            if aggmode == "full":
                                # batched one-hot build: S_all[p, k, d] =
                                # (ramp[p, d] == dest_rel[p, q0+k])
                                S = pool.tile([128, nk, 128], bf16, tag="S")
                                ramp_b = rampb[:].unsqueeze(1).broadcast_to(
                                    [128, nk, 128])
                                dr_b = drt[:, q0:q0 + nk].unsqueeze(2).broadcast_to(
                                    [128, nk, 128])
                                nc.vector.tensor_tensor(S[:], ramp_b, dr_b,
                                                        OP.is_equal)
                            for k in range(nk):
                                slot = q0 + k - qs
                                lhs = rampb[:] if aggmode == "noS" else S[:, k, :]
                                nc.tensor.matmul(ps[:], lhs, gb[:, slot, 0:Fl],
                                                 start=(k == 0), stop=(k == nk - 1),
                                                 skip_group_check=True)
                            nc.vector.tensor_tensor(
                                acc[:, ti * Fl:(ti + 1) * Fl],
                                acc[:, ti * Fl:(ti + 1) * Fl], ps[:], OP.add)
                            if tile_cb is not None and \
                                    (wi, ti, q0) in final_piece:
                                tile_cb(ti, *cb_pools)

            def aggregate_sb(wviews, Fl, acc):
                # SBUF-source gather: stage each 32768-token window of the
                # bf16 table into SBUF (bulk DMA), then transpose-mode
                # dma_gather (SBUF source avoids the HBM random-read
                # latency; descriptor gen round-robins 4 SWDGE queues).
                # Output is feature-major [token_elem, edge]; a per-chunk
                # PE transpose restores edge-major for the scatter matmul.
                RKS = WIN // 128  # 256 ranks per full window
                with tc.tile_pool(name="agw", bufs=1) as wpool, \
                     tc.tile_pool(name="agg", bufs=3) as pool, \
                     tc.tile_pool(name="aggi", bufs=4) as ipool, \
                     tc.tile_pool(name="aggs", bufs=2) as spool, \
                     tc.tile_pool(name="aggt", bufs=3) as tpool, \
                     tc.tile_pool(name="aggp", bufs=4, space="PSUM") as pp, \
                     tc.tile_pool(name="aggq", bufs=4, space="PSUM") as pq:
                    cur_w = -1
                    win = None
                    for ci, (wi, qs, nq, segs) in enumerate(calls):
                        if wi != cur_w:
                            cur_w = wi
                            nrk = _win_len(wi) // 128
                            win = wpool.tile([128, RKS, 128], bf16, tag="win")
                            nc.sync.dma_start(
                                out=win[:, 0:nrk, :],
                                in_=wviews[wi][:, 0:nrk, :])
                        it = ipool.tile([128, nq * 8], i16, tag="it")
                        nc.sync.dma_start(out=it[:],
                                          in_=idxw[:, qs * 8:(qs + nq) * 8])
                        gb = pool.tile([128, 1, CALL_CHUNKS * 128], bf16,
                                       tag="gb")
                        nc.gpsimd.dma_gather(
                            gb[:, :, 0:nq * 128], win[:],
                            it[:], nq * 128, nq * 128, 128,
                            transpose=True, single_packet=False,
                            queue_num=ci % 4,
                            sbuf_tokens_per_rank=128,
                            sbuf_free_dim_per_rank=256,
                        )
                        if aggmode == "gather":
                            continue
                        for (ti, q0, nk) in segs:
                            ps = pp.tile([128, Fl], f32, tag="ps")
                            if aggmode == "full":
                                S = spool.tile([128, nk, 128], bf16, tag="S")
                                ramp_b = rampb[:].unsqueeze(1).broadcast_to(
                                    [128, nk, 128])
                                dr_b = drt[:, q0:q0 + nk].unsqueeze(2).broadcast_to(
                                    [128, nk, 128])
                                nc.vector.tensor_tensor(S[:], ramp_b, dr_b,
                                                        OP.is_equal)
                            for k in range(nk):
                                slot = q0 + k - qs
                                tp = pq.tile([128, 128], bf16, tag="tp")
                                nc.tensor.transpose(
                                    tp[:], gb[:, 0, slot * 128:(slot + 1) * 128],
                                    identb[:])
                                ts = tpool.tile([128, Fl], bf16, tag="ts")
                                nc.scalar.activation(ts[:], tp[:, 0:Fl], AF.Copy)
                                lhs = rampb[:] if aggmode == "noS" else S[:, k, :]
                                nc.tensor.matmul(ps[:], lhs, ts[:],
                                                 start=(k == 0),
                                                 stop=(k == nk - 1),
                                                 skip_group_check=True)
                            nc.vector.tensor_tensor(
                                acc[:, ti * Fl:(ti + 1) * Fl],
                                acc[:, ti * Fl:(ti + 1) * Fl], ps[:], OP.add)

            def d_tile(t, pool, pp, pq):
                # h1 = relu(dis*acc1); tab2_local[d] = dis_d*(h1_d @ W2.T + b2)
                # (also the layer-2 self-loop init for acc2)
                rl = pool.tile([128, H1], f32, tag="rl")
                nc.scalar.activation(rl[:], acc1[:, t * H1:(t + 1) * H1],
                                     AF.Relu, scale=disl[:, t:t + 1])
                pT = pp.tile([H1, 128], f32, tag="pT")
                nc.tensor.transpose(pT[:], rl[:], ident[:, :])
                rT = pool.tile([H1, 128], f32, tag="rT")
                nc.scalar.activation(rT[:], pT[:], AF.Copy)
                p2 = pp.tile([H2, 128], f32, tag="p2")
                nc.tensor.matmul(p2[:], w2s[:], rT[:], start=True, stop=True)
                hb2 = pool.tile([H2, 128], f32, tag="hb2")
                nc.vector.tensor_scalar(hb2[:], p2[:], b2s[:], None, OP.add)
                pj2 = pq.tile([128, H2], f32, tag="pj2")
                nc.tensor.transpose(pj2[:], hb2[:], ident[:H2, :H2])
                nc.vector.tensor_scalar(acc2[:, t * H2:(t + 1) * H2],
                                        pj2[:], disl[:, t:t + 1], None, OP.mult)
                tb2 = pool.tile([128, H2], bf16, tag="tb2")
                nc.vector.tensor_copy(tb2[:], acc2[:, t * H2:(t + 1) * H2])
                nc.sync.dma_start(out=tab2inv[:, t, 0:H2], in_=tb2[:])

            def tab2_allgather():
                nc.gpsimd.collective_compute(
                    "AllGather", mybir.AluOpType.bypass, replica_groups=rg,
                    ins=[tab2in.ap().opt()], outs=[tab2.ap().opt()])

            def build_tab2_local_allgather():
                with tc.tile_pool(name="rt", bufs=3) as pool, \
                     tc.tile_pool(name="rtp", bufs=2, space="PSUM") as pp, \
                     tc.tile_pool(name="rtq", bufs=2, space="PSUM") as pq:
                    for t in range(TLOC):
                        d_tile(t, pool, pp, pq)
                tab2_allgather()

            def g_tile(t, pool, pps, first, last):
                # pooled-sum contribution of dest tile t (per-graph one-hot)
                r2 = pool.tile([128, H2], f32, tag="r2")
                nc.scalar.activation(r2[:], acc2[:, t * H2:(t + 1) * H2],
                                     AF.Relu, scale=disl[:, t:t + 1])
                Sb = pool.tile([128, NG], f32, tag="Sb")
                nc.vector.tensor_scalar(Sb[:], ramp[:, 0:NG],
                                        bos[:, t:t + 1], None, OP.is_equal)
                nc.tensor.matmul(pps[:], Sb[:], r2[:],
                                 start=first, stop=last,
                                 skip_group_check=True)

            # callback emission order (order in which tiles finalize)
            cb_order = [ti for (wi, ti, q0, nk) in segments
                        if (wi, ti, q0) in final_piece]
            assert len(cb_order) == TLOC

            def pool_and_head(pps=None):
                with tc.tile_pool(name="hd", bufs=3) as pool, \
                     tc.tile_pool(name="hdp", bufs=1, space="PSUM") as pp:
                    if pps is None:
                        pps = pp.tile([NG, H2], f32, tag="pool")
                        for t in range(TLOC):
                            g_tile(t, pool, pps, t == 0, t == TLOC - 1)
                    pls = pool.tile([NG, H2], f32, tag="pls")
                    nc.scalar.activation(pls[:], pps[:], AF.Copy)
                    nc.sync.dma_start(out=cc2_in[:, :], in_=pls[:])
                    nc.gpsimd.collective_compute(
                        "AllReduce", OP.add, replica_groups=rg,
                        ins=[cc2_in.ap().opt()], outs=[cc2_out.ap().opt()])
                    psb = pool.tile([NG, H2], f32, tag="psb")
                    nc.sync.dma_start(out=psb[:], in_=cc2_out[:, :])
                    rc = pool.tile([NG, 1], f32, tag="rc")
                    nc.vector.reciprocal(rc[:], cnts[:])
                    mean = pool.tile([NG, H2], f32, tag="mean")
                    nc.vector.tensor_scalar(mean[:], psb[:], rc[:], None, OP.mult)
                    # transpose mean -> [H2, NG]
                    pmT = pp.tile([H2, NG], f32, tag="pmT")
                    nc.tensor.transpose(pmT[:], mean[:], ident[:NG, :NG])
                    meanT = pool.tile([H2, NG], f32, tag="meanT")
                    nc.scalar.activation(meanT[:], pmT[:], AF.Copy)
                    # logitsT [NCLS, NG]
                    plt = pp.tile([NCLS, NG], f32, tag="plt")
                    nc.tensor.matmul(plt[:], wfs[:], meanT[:], start=True, stop=True)
                    lts = pool.tile([NCLS, NG], f32, tag="lts")
                    nc.vector.tensor_scalar(lts[:], plt[:], bfs[:], None, OP.add)
                    # transpose -> [NG, NCLS]
                    plg = pp.tile([NG, NCLS], f32, tag="plg")
                    nc.tensor.transpose(plg[:], lts[:], ident[:NCLS, :NCLS])
                    lg = pool.tile([NG, NCLS], f32, tag="lg")
                    nc.scalar.activation(lg[:], plg[:], AF.Copy)
                    # softmax over free dim
                    mx = pool.tile([NG, 1], f32, tag="mx")
                    nc.vector.tensor_reduce(mx[:], lg[:], mybir.AxisListType.X,
                                            OP.max, negate=True)
                    ex = pool.tile([NG, NCLS], f32, tag="ex")
                    nc.scalar.activation(ex[:], lg[:], AF.Exp, bias=mx[:])
                    sm = pool.tile([NG, 1], f32, tag="sm")
                    nc.vector.tensor_reduce(sm[:], ex[:], mybir.AxisListType.X,
                                            OP.add)
                    rs = pool.tile([NG, 1], f32, tag="rs")
                    nc.vector.reciprocal(rs[:], sm[:])
                    yt = pool.tile([NG, NCLS], f32, tag="yt")
                    nc.vector.tensor_scalar(yt[:], ex[:], rs[:], None, OP.mult)
                    nc.sync.dma_start(out=y[:, :], in_=yt[:])

            sbg = os.environ.get("GNN_SBGATHER", "0") == "1"
            di = os.environ.get("GNN_DI", "1") == "1"
            sv2 = [tab2v[:, w * 256:w * 256 + _win_len(w) // 128, :]
                   for w in range(NWIN)]
            if "B" in stages:
                build_table1()
            interleaved_d = di and not sbg and "C" in stages and "D" in stages
            if "C" in stages:
                init_acc1()
                if sbg:
                    aggregate_sb(tab1wv, H1, acc1)
                elif interleaved_d:
                    # fold the layer-2 table build into the aggregation: each
                    # dest tile's tab2 row block is computed as soon as its
                    # accumulator finalizes, hiding D behind the gathers.
                    with tc.tile_pool(name="rt", bufs=3) as dpool, \
                         tc.tile_pool(name="rtp", bufs=1, space="PSUM") as dpp, \
                         tc.tile_pool(name="rtq", bufs=1, space="PSUM") as dpq:
                        aggregate(src1, H1, acc1, tile_cb=d_tile,
                                  cb_pools=(dpool, dpp, dpq))
                    tab2_allgather()
                else:
                    aggregate(src1, H1, acc1)
            if "D" in stages and not interleaved_d:
                build_tab2_local_allgather()
            gi = os.environ.get("GNN_GI", "1") == "1"
            interleaved_g = gi and not sbg and "F" in stages and "G" in stages \
                and aggmode == "full"
            gpps = None
            if "F" in stages:
                if sbg:
                    aggregate_sb(sv2, H2, acc2)
                elif interleaved_g:
                    # fold the graph-pooling matmul into the aggregation:
                    # each dest tile is pooled as soon as it finalizes, so
                    # only the AllReduce + head remain after the gathers.
                    with tc.tile_pool(name="gp", bufs=1, space="PSUM") as gpp, \
                         tc.tile_pool(name="gs", bufs=2) as gspool:
                        gpps = gpp.tile([NG, H2], f32, tag="pool")
                        cb_first, cb_last = cb_order[0], cb_order[-1]

                        def _gcb(ti, pool, pps):
                            g_tile(ti, pool, pps, ti == cb_first, ti == cb_last)

                        aggregate(src2, H2, acc2, tile_cb=_gcb,
                                  cb_pools=(gspool, gpps))
                        pool_and_head(gpps)
                else:
                    aggregate(src2, H2, acc2)
            if "G" in stages:
                if not interleaved_g:
                    pool_and_head()
            else:
                with tc.tile_pool(name="dbg", bufs=1) as dpool:
                    dt = dpool.tile([NG, NCLS], f32)
                    nc.vector.memset(dt[:], 0.0)
                    nc.sync.dma_start(out=y[:, :], in_=dt[:])

    nc.compile()
    return nc


def kernel(x, edge_index, batch, W1, b1, W2, b2, Wf, bf):
    from concourse.bass_utils import run_bass_kernel_spmd

    prep = _host_prep(x, edge_index, batch)
    nc = _build_program(prep["NCH"], prep["segments"], prep["calls"])
    in_maps = build_in_maps(prep, W1, b1, W2, b2, Wf, bf)
    res = run_bass_kernel_spmd(nc, in_maps, core_ids=list(range(NCORES)))
    return np.asarray(res.results[0]["y"], np.float32)
